# revision 32
# baseline (speedup 1.0000x reference)
"""Distributed Trainium2 Bass kernel for the single-step attention decoder.

Strategy (8-way tensor parallel, memory-bound matvec workload):
  - emb_table[x] gathered on host (only 1 row of 411MB is needed).
  - attn_W and encoder_outputs replicated (small); comb_W row-sharded (256
    of 2048 output dims); W_ih/W_hh contraction-sharded (aligned with comb's
    row shard); fc_W vocab-sharded (6400 padded rows/core).
  - 2 AllGathers: partial GRU gate pre-activations, and per-core
    (max, sumexp) stats for the distributed log-softmax.
  - Matvecs run on the TensorEngine with the vector chunk stationary and the
    weight tile moving, except fc, where the weight tile is stationary so
    the logits land partition-major for a cheap parallel softmax.
  - Weight shards are pre-transposed on the host so every device DMA is a
    large contiguous burst with the contraction dim on SBUF partitions.
  - Partition-axis relayouts (vector [N] <-> SBUF [128, N/128]) go through
    PE transposes; partition-scatter DMA patterns run at ~3GB/s and are
    avoided everywhere.
  - DMA queue discipline: each engine's DMA stream is FIFO and blocks on
    pool-slot waits, so the fc weight stream gets the sync queue to itself
    (prefetches from t=0); all chain weights go in chain order on the
    scalar queue.
"""
import os
import sys
import numpy as np

for _p in ("/opt/trn_rl_repo",):
    if _p not in sys.path:
        sys.path.append(_p)

V, H, L = 50257, 2048, 512
NCORES = 8
VS = 6400            # per-core padded vocab rows = 128 * 50
VT = VS // 128       # 50 vocab tiles per core
HC = H // 128        # 16 hidden chunks
SH = H // NCORES     # 256: comb row shard / gru contraction shard
G3 = 3 * H           # 6144
GT = G3 // 128       # 48 gate tiles
NEG = -1.0e30

USE_BF16 = os.environ.get("ADK_BF16", "1") == "1"

_CACHE = {}


def _build(use_bf16):
    from concourse import bacc, mybir, tile

    dt = mybir.dt
    f32 = dt.float32
    wdt = dt.bfloat16 if use_bf16 else f32
    A = mybir.AluOpType
    AF = mybir.ActivationFunctionType
    X = mybir.AxisListType.X

    nc = bacc.Bacc("TRN2", target_bir_lowering=False, debug=False,
                   enable_asserts=True, num_devices=NCORES)

    def din(name, shape, d=f32):
        return nc.dram_tensor(name, list(shape), d, kind="ExternalInput")

    emb = din("emb", [H])
    h0 = din("h0", [H])
    h0s = din("h0s", [SH])
    attn_wt = din("attn_wt", [2 * H, L], wdt)   # replicated, [j, L]
    attn_b = din("attn_b", [L])
    enc = din("enc", [L, H], wdt)               # replicated
    comb_wt = din("comb_wt", [2 * H, SH], wdt)  # [j, h-shard]
    comb_b = din("comb_b", [SH])
    wih_t = din("wih_t", [SH, G3], wdt)
    whh_t = din("whh_t", [SH, G3], wdt)
    b_ih = din("b_ih", [G3])
    b_hh = din("b_hh", [G3])
    fc_wt = din("fc_wt", [H, VS], wdt)          # [h, v-shard]
    fc_b = din("fc_b", [VS])

    out_logits = nc.dram_tensor("out_logits", [VS], f32, kind="ExternalOutput")
    out_h1 = nc.dram_tensor("out_h1", [H], f32, kind="ExternalOutput")
    out_aw = nc.dram_tensor("out_aw", [L], f32, kind="ExternalOutput")

    ident_np = np.eye(128, dtype=np.float32)
    rg = [list(range(NCORES))]

    with tile.TileContext(nc) as tc:
        with (
            tc.tile_pool(name="small", bufs=1) as sp,
            tc.tile_pool(name="wpool", bufs=1) as wp,
            tc.tile_pool(name="ps", bufs=3, space="PSUM") as ps,
            tc.tile_pool(name="psc", bufs=3, space="PSUM") as psc,
            tc.tile_pool(name="psl", bufs=1, space="PSUM") as psl,
            tc.tile_pool(name="pss", bufs=1, space="PSUM") as pss,
            tc.tile_pool(name="dram", bufs=1, space="DRAM") as dp,
        ):
            # ---------- constants ----------
            id128 = sp.tile([128, 128], f32, tag="id128")
            id_dram = nc.inline_tensor(ident_np, name="id_dram")
            nc.scalar.dma_start(id128[:], id_dram[:])
            ones128 = sp.tile([128, 1], f32, tag="ones128")
            nc.vector.memset(ones128[:], 1.0)
            onesrow = sp.tile([1, 128], f32, tag="onesrow")
            nc.vector.memset(onesrow[:], 1.0)

            # ---------- fc weight stream: alone on the sync queue so it
            # prefetches from t=0 with nothing blocking in front of it
            fc_bufs = 7 if use_bf16 else 2
            fc_tiles = []
            for c in range(HC):
                wc = wp.tile([128, VS], wdt, tag="fc", bufs=fc_bufs,
                             name=f"fcw{c}")
                for dd in range(2):
                    nc.sync.dma_start(wc[:, dd * 3200:(dd + 1) * 3200],
                                      fc_wt[c * 128:(c + 1) * 128,
                                            dd * 3200:(dd + 1) * 3200])
                fc_tiles.append(wc)

            def load_T(src_ap, cols, tag, eng=None):
                """DRAM vector [cols*128] -> SBUF [128, cols] via natural
                load + PE transpose (no partition-scatter DMA)."""
                eng = eng or nc.scalar
                nat = wp.tile([cols, 128], f32, tag="ldT_nat", bufs=4,
                              name="ldT_" + tag)
                eng.dma_start(nat[:], src_ap.rearrange("(c p) -> c p", p=128))
                tp = psc.tile([128, 128], f32, tag="col", name="tp_" + tag)
                nc.tensor.transpose(tp[:, 0:cols], nat[:], id128[:cols, :cols])
                t = sp.tile([128, cols], f32, tag=tag, name=tag)
                nc.vector.tensor_copy(t[:], tp[:, 0:cols])
                return t

            def store_T(dst_ap, src_tile, cols, eng=None):
                """SBUF [128, cols] -> DRAM vector [cols*128] via PE
                transpose + natural store."""
                eng = eng or nc.scalar
                tp = psc.tile([128, 128], f32, tag="col", name="tp_st")
                nc.tensor.transpose(tp[:cols, :], src_tile[:], id128[:])
                nat = wp.tile([cols, 128], f32, tag="stT_nat", bufs=2,
                              name="stT_nat")
                nc.vector.tensor_copy(nat[:], tp[:cols, :])
                eng.dma_start(dst_ap.rearrange("(c p) -> c p", p=128), nat[:])

            attnb_sb = sp.tile([1, L], f32, tag="attnb_sb")
            nc.scalar.dma_start(attnb_sb[:], attn_b[None, :])
            emb_sb = load_T(emb[:], HC, "emb_sb")
            h0_sb = load_T(h0[:], HC, "h0_sb")
            h0s_sb = load_T(h0s[:], 2, "h0s_sb")

            if use_bf16:
                emb_w = sp.tile([128, HC], wdt, tag="emb_w")
                nc.vector.tensor_copy(emb_w[:], emb_sb[:])
                h0_w = sp.tile([128, HC], wdt, tag="h0_w")
                nc.vector.tensor_copy(h0_w[:], h0_sb[:])
                h0s_w = sp.tile([128, 2], wdt, tag="h0s_w")
                nc.vector.tensor_copy(h0s_w[:], h0s_sb[:])
            else:
                emb_w, h0_w, h0s_w = emb_sb, h0_sb, h0s_sb

            # softmax helper used by the final distributed log-softmax
            def pmax_bcast_neg(val_pc, tag):
                """[128,1] per-partition vals -> (max, [128,1] bcast of -max)."""
                tp = pss.tile([1, 128], f32, tag="tp", name="tp_" + tag)
                nc.tensor.transpose(tp[:], val_pc[:], id128[:])
                m = sp.tile([1, 1], f32, tag=tag + "_m")
                nc.vector.tensor_reduce(m[:], tp[:], axis=X, op=A.max)
                negm = sp.tile([1, 1], f32, tag=tag + "_negm")
                nc.scalar.mul(negm[:], m[:], -1.0)
                bc_ps = psc.tile([128, 128], f32, tag="col", name="bc_" + tag)
                nc.tensor.matmul(bc_ps[:, 0:1], onesrow[:], negm[:])
                bc = sp.tile([128, 1], f32, tag=tag + "_bc")
                nc.vector.tensor_copy(bc[:], bc_ps[:, 0:1])
                return m, bc

            # ---------- attention scores, fully replicated (no collective)
            att_ps = ps.tile([1, L], f32, tag="v512", name="att_ps")
            for j in range(32):
                wt = wp.tile([128, L], wdt, tag="attn_wt", bufs=8,
                             name=f"attw{j}")
                nc.scalar.dma_start(wt[:], attn_wt[j * 128:(j + 1) * 128, :])
                lhs = emb_w[:, j:j + 1] if j < HC else h0_w[:, j - HC:j - HC + 1]
                nc.tensor.matmul(att_ps[:], lhs, wt[:],
                                 start=(j == 0), stop=(j == 31))

            # enc + gru weight loads issued now on the chain queue (they
            # depend on nothing, and nothing before them can block)
            enc_tiles = []
            for k in range(4):
                et = wp.tile([128, H], wdt, tag="enc", bufs=4, name=f"enc{k}")
                nc.scalar.dma_start(et[:], enc[k * 128:(k + 1) * 128, :])
                enc_tiles.append(et)
            gru_tiles = {}
            for nm, wten in (("hh", whh_t), ("ih", wih_t)):
                for ck in range(2):
                    gwt = wp.tile([128, G3], wdt, tag="gru", bufs=4,
                                  name=f"gru_{nm}{ck}")
                    nc.scalar.dma_start(gwt[:], wten[ck * 128:(ck + 1) * 128, :])
                    gru_tiles[(nm, ck)] = gwt

            # single-lane softmax over [1, 512]
            sc_row = sp.tile([1, L], f32, tag="sc_row")
            nc.vector.tensor_add(sc_row[:], att_ps[:], attnb_sb[:])
            smax = sp.tile([1, 1], f32, tag="smax")
            nc.vector.tensor_reduce(smax[:], sc_row[:], axis=X, op=A.max)
            nsmax = sp.tile([1, 1], f32, tag="nsmax")
            nc.scalar.mul(nsmax[:], smax[:], -1.0)
            aw_row = sp.tile([1, L], f32, tag="aw_row")
            ssum = sp.tile([1, 1], f32, tag="ssum")
            nc.scalar.activation(aw_row[:], sc_row[:], AF.Exp, bias=nsmax[:],
                                 accum_out=ssum[:])
            rsum = sp.tile([1, 1], f32, tag="rsum")
            nc.vector.reciprocal(rsum[:], ssum[:])
            nc.vector.tensor_scalar(aw_row[:], aw_row[:], rsum[:], None,
                                    op0=A.mult)
            nc.scalar.dma_start(out_aw[None, :], aw_row[:])

            # attn weights -> partition-major [128, 4] for the applied matvec
            awT_ps = psc.tile([128, 128], f32, tag="col", name="awT")
            for t in range(4):
                nc.tensor.transpose(awT_ps[:, t:t + 1],
                                    aw_row[:, t * 128:(t + 1) * 128],
                                    id128[:1, :1])
            aw_sb = sp.tile([128, 4], f32, tag="aw_sb")
            nc.vector.tensor_copy(aw_sb[:], awT_ps[:, 0:4])
            if use_bf16:
                aw_w = sp.tile([128, 4], wdt, tag="aw_w")
                nc.vector.tensor_copy(aw_w[:], aw_sb[:])
            else:
                aw_w = aw_sb

            # ---------- GRU partials: the hh half depends only on h0s, so
            # it runs while the attention chain is still in flight
            gp_in = dp.tile([1, 2 * G3], f32, tag="gp_in")
            gp_out = dp.tile([NCORES, 2 * G3], f32, tag="gp_out")

            def gru_half(nm, xw, base):
                for s in range(12):
                    gps = ps.tile([1, 512], f32, tag="v512", name=f"g{nm}{s}")
                    nc.tensor.matmul(gps[:], xw[:, 0:1],
                                     gru_tiles[(nm, 0)][:, s * 512:(s + 1) * 512],
                                     start=True, stop=False)
                    nc.tensor.matmul(gps[:], xw[:, 1:2],
                                     gru_tiles[(nm, 1)][:, s * 512:(s + 1) * 512],
                                     start=False, stop=True)
                    off = base + s * 512
                    stg = wp.tile([1, 512], f32, tag="gpstage", bufs=4,
                                  name=f"stg{nm}{s}")
                    nc.vector.tensor_copy(stg[:], gps[:])
                    nc.gpsimd.dma_start(gp_in[:, off:off + 512], stg[:])

            gru_half("hh", h0s_w, G3)

            # ---------- attn_applied = attn_weight @ enc  (full, replicated)
            app_row = sp.tile([1, H], f32, tag="app_row")
            for s in range(4):
                app_ps = ps.tile([1, 512], f32, tag="v512", name=f"app{s}")
                for k in range(4):
                    nc.tensor.matmul(app_ps[:], aw_w[:, k:k + 1],
                                     enc_tiles[k][:, s * 512:(s + 1) * 512],
                                     start=(k == 0), stop=(k == 3))
                nc.vector.tensor_copy(app_row[:, s * 512:(s + 1) * 512],
                                      app_ps[:])

            # transpose [1, 2048] -> [128, 16]
            appT_ps = psc.tile([128, 128], f32, tag="col", name="appT")
            for t in range(HC):
                nc.tensor.transpose(appT_ps[:, t:t + 1],
                                    app_row[:, t * 128:(t + 1) * 128],
                                    id128[:1, :1])
            app_sb = sp.tile([128, HC], f32, tag="app_sb")
            nc.vector.tensor_copy(app_sb[:], appT_ps[:, 0:HC])
            if use_bf16:
                app_w = sp.tile([128, HC], wdt, tag="app_w")
                nc.vector.tensor_copy(app_w[:], app_sb[:])
            else:
                app_w = app_sb

            # ---------- comb: g_in shard [1, 256] = relu(comb_in @ W.T + b)
            combb_sb = sp.tile([1, SH], f32, tag="combb_sb")
            nc.scalar.dma_start(combb_sb[:], comb_b[None, :])
            comb_ps = ps.tile([1, SH], f32, tag="v512", name="comb_ps")
            for j in range(32):
                ct = wp.tile([128, SH], wdt, tag="comb", bufs=6,
                             name=f"combw{j}")
                nc.scalar.dma_start(ct[:], comb_wt[j * 128:(j + 1) * 128, :])
                lhs = emb_w[:, j:j + 1] if j < HC else app_w[:, j - HC:j - HC + 1]
                nc.tensor.matmul(comb_ps[:], lhs, ct[:],
                                 start=(j == 0), stop=(j == 31))
            gin_row = sp.tile([1, SH], f32, tag="gin_row")
            nc.vector.scalar_tensor_tensor(gin_row[:], comb_ps[:], 1.0,
                                           combb_sb[:], op0=A.mult, op1=A.add)
            nc.scalar.activation(gin_row[:], gin_row[:], AF.Relu)
            gT_ps = psc.tile([128, 128], f32, tag="col", name="gT")
            for t in range(2):
                nc.tensor.transpose(gT_ps[:, t:t + 1],
                                    gin_row[:, t * 128:(t + 1) * 128],
                                    id128[:1, :1])
            gin_sb = sp.tile([128, 2], f32, tag="gin_sb")
            nc.vector.tensor_copy(gin_sb[:], gT_ps[:, 0:2])
            if use_bf16:
                gin_w = sp.tile([128, 2], wdt, tag="gin_w")
                nc.vector.tensor_copy(gin_w[:], gin_sb[:])
            else:
                gin_w = gin_sb

            # ---------- GRU ih partials, then gather
            gru_half("ih", gin_w, 0)

            nc.gpsimd.collective_compute(
                "AllGather", mybir.AluOpType.bypass, replica_groups=rg,
                ins=[gp_in.opt()], outs=[gp_out.opt()])

            # bias relayout (off critical path)
            bias_sb = sp.tile([128, 2 * GT], f32, tag="bias_sb")
            bi = load_T(b_ih[:], GT, "bi_tmp")
            bh = load_T(b_hh[:], GT, "bh_tmp")
            nc.vector.tensor_copy(bias_sb[:, 0:GT], bi[:])
            nc.vector.tensor_copy(bias_sb[:, GT:2 * GT], bh[:])

            # relayout gathered partials: flat [8*12288] = [768, 128]
            # -> 6 natural [128,128] tiles -> PE transpose -> col = r*96 + j
            W2 = 2 * GT
            gg2 = sp.tile([128, NCORES * W2], f32, tag="gg2")
            gpf = gp_out[:].rearrange("r (k p) -> (r k) p", p=128)
            for b in range(6):
                gnat = wp.tile([128, 128], f32, tag="gg_nat", bufs=3,
                               name=f"gg_nat{b}")
                nc.scalar.dma_start(gnat[:], gpf[b * 128:(b + 1) * 128, :])
                ggT = psc.tile([128, 128], f32, tag="col", name=f"ggT{b}")
                nc.tensor.transpose(ggT[:], gnat[:], id128[:])
                nc.vector.tensor_copy(gg2[:, b * 128:(b + 1) * 128], ggT[:])
            gsum = sp.tile([128, W2], f32, tag="gsum")
            nc.vector.tensor_add(gsum[:], gg2[:, 0:W2], gg2[:, W2:2 * W2])
            for r in range(2, NCORES):
                nc.vector.tensor_add(gsum[:], gsum[:],
                                     gg2[:, r * W2:(r + 1) * W2])
            nc.vector.tensor_add(gsum[:], gsum[:], bias_sb[:])

            # gates: [ir iz in | hr hz hn] at 16-col blocks
            ir, iz, inn = gsum[:, 0:16], gsum[:, 16:32], gsum[:, 32:48]
            hr, hz, hn = gsum[:, 48:64], gsum[:, 64:80], gsum[:, 80:96]
            r_sb = sp.tile([128, HC], f32, tag="r_sb")
            nc.vector.tensor_add(r_sb[:], ir, hr)
            nc.scalar.activation(r_sb[:], r_sb[:], AF.Sigmoid)
            z_sb = sp.tile([128, HC], f32, tag="z_sb")
            nc.vector.tensor_add(z_sb[:], iz, hz)
            nc.scalar.activation(z_sb[:], z_sb[:], AF.Sigmoid)
            n_sb = sp.tile([128, HC], f32, tag="n_sb")
            nc.vector.tensor_mul(n_sb[:], r_sb[:], hn)
            nc.vector.tensor_add(n_sb[:], n_sb[:], inn)
            nc.scalar.activation(n_sb[:], n_sb[:], AF.Tanh)
            h1_sb = sp.tile([128, HC], f32, tag="h1_sb")
            nc.vector.tensor_sub(h1_sb[:], h0_sb[:], n_sb[:])
            nc.vector.tensor_mul(h1_sb[:], z_sb[:], h1_sb[:])
            nc.vector.tensor_add(h1_sb[:], n_sb[:], h1_sb[:])
            store_T(out_h1[:], h1_sb, HC)
            if use_bf16:
                h1_w = sp.tile([128, HC], wdt, tag="h1_w")
                nc.vector.tensor_copy(h1_w[:], h1_sb[:])
            else:
                h1_w = h1_sb

            # ---------- fc: logits [128, 50] vocab shard
            fcb_sb = load_T(fc_b[:], VT, "fcb_sb")
            lg_ps = psl.tile([128, VT], f32, tag="lg")
            for c in range(HC):
                wc = fc_tiles[c]
                for t in range(VT):
                    # NOTE: start=True clears the accumulate-state of the
                    # whole PSUM bank, so only the very first matmul of the
                    # fc phase may set it; each column's first start=False
                    # write then overwrites (bits clear) and later ones add.
                    nc.tensor.matmul(lg_ps[:, t:t + 1],
                                     wc[:, t * 128:(t + 1) * 128],
                                     h1_w[:, c:c + 1],
                                     start=(c == 0 and t == 0),
                                     stop=(c == HC - 1 and t == VT - 1),
                                     skip_group_check=True)

            lg_sb = sp.tile([128, VT], f32, tag="lg_sb")
            nc.vector.tensor_add(lg_sb[:], lg_ps[:], fcb_sb[:])

            # local softmax stats
            rmax2 = sp.tile([128, 1], f32, tag="rmax2")
            nc.vector.tensor_reduce(rmax2[:], lg_sb[:], axis=X, op=A.max)
            m_sb, nm2_bc = pmax_bcast_neg(rmax2, "fc")
            ex2 = sp.tile([128, VT], f32, tag="ex2")
            se2 = sp.tile([128, 1], f32, tag="se2")
            nc.scalar.activation(ex2[:], lg_sb[:], AF.Exp, bias=nm2_bc[:],
                                 accum_out=se2[:])
            s_ps = pss.tile([1, 128], f32, tag="tp", name="s_ps")
            nc.tensor.matmul(s_ps[:1, 0:1], se2[:], ones128[:])
            stats_sb = sp.tile([1, 2], f32, tag="stats_sb")
            nc.scalar.copy(stats_sb[:, 0:1], m_sb[:])
            nc.scalar.copy(stats_sb[:, 1:2], s_ps[:1, 0:1])

            st_in = dp.tile([1, 2], f32, tag="st_in")
            st_out = dp.tile([NCORES, 2], f32, tag="st_out")
            nc.scalar.dma_start(st_in[:], stats_sb[:])
            nc.gpsimd.collective_compute(
                "AllGather", mybir.AluOpType.bypass, replica_groups=rg,
                ins=[st_in.opt()], outs=[st_out.opt()])
            sts = sp.tile([1, NCORES, 2], f32, tag="sts")
            nc.scalar.dma_start(sts[:], st_out[None, :, :])

            # transpose the biased logits while the stats AllGather runs
            lgT_ps = psc.tile([128, 128], f32, tag="col", name="lgT_ps")
            nc.tensor.transpose(lgT_ps[:VT, :], lg_sb[:], id128[:])
            lgT = sp.tile([VT, 128], f32, tag="lgT")
            nc.vector.tensor_copy(lgT[:], lgT_ps[:VT, :])

            gm = sp.tile([1, 1], f32, tag="gm")
            nc.vector.tensor_reduce(gm[:], sts[:, :, 0], axis=X, op=A.max)
            ngm = sp.tile([1, 1], f32, tag="ngm")
            nc.scalar.mul(ngm[:], gm[:], -1.0)
            ee = sp.tile([1, NCORES], f32, tag="ee")
            nc.scalar.activation(ee[:], sts[:, :, 0], AF.Exp, bias=ngm[:])
            nc.vector.tensor_mul(ee[:], ee[:], sts[:, :, 1])
            gs = sp.tile([1, 1], f32, tag="gs")
            nc.vector.tensor_reduce(gs[:], ee[:], axis=X, op=A.add)
            lgs = sp.tile([1, 1], f32, tag="lgs")
            nc.scalar.activation(lgs[:], gs[:], AF.Ln)
            cv = sp.tile([1, 1], f32, tag="cv")
            nc.vector.tensor_add(cv[:], gm[:], lgs[:])
            ncv = sp.tile([1, 1], f32, tag="ncv")
            nc.scalar.mul(ncv[:], cv[:], -1.0)
            ncv_ps = psc.tile([128, 128], f32, tag="col", name="ncv_ps")
            nc.tensor.matmul(ncv_ps[:VT, 0:1], onesrow[:1, 0:VT], ncv[:])
            ncv_bc = sp.tile([VT, 1], f32, tag="ncv_bc")
            nc.vector.tensor_copy(ncv_bc[:], ncv_ps[:VT, 0:1])
            fin = sp.tile([VT, 128], f32, tag="fin")
            nc.vector.tensor_scalar(fin[:], lgT[:], ncv_bc[:], None,
                                    op0=A.add)
            nc.sync.dma_start(out_logits[:].rearrange("(c p) -> c p", p=128),
                              fin[:])

    nc.compile()
    return nc


def _get_nc(use_bf16):
    if use_bf16 not in _CACHE:
        _CACHE[use_bf16] = _build(use_bf16)
    return _CACHE[use_bf16]


def _prep_in_maps(inputs, use_bf16):
    import ml_dtypes
    wnp = ml_dtypes.bfloat16 if use_bf16 else np.float32

    def wcast(a):
        return np.ascontiguousarray(a, dtype=np.float32).astype(wnp) \
            if use_bf16 else np.ascontiguousarray(a, dtype=np.float32)

    x = np.asarray(inputs["x"]).reshape(-1)
    idx = int(x[0])
    emb_tab = np.asarray(inputs["emb_table"], dtype=np.float32)
    emb_row = np.ascontiguousarray(emb_tab[idx])
    h0f = np.asarray(inputs["hidden"], dtype=np.float32).reshape(H)
    attn_W = np.asarray(inputs["attn_W"], dtype=np.float32)
    attn_b = np.ascontiguousarray(np.asarray(inputs["attn_b"], np.float32))
    enc = np.asarray(inputs["encoder_outputs"], dtype=np.float32)
    comb_W = np.asarray(inputs["comb_W"], dtype=np.float32)
    comb_b = np.asarray(inputs["comb_b"], dtype=np.float32)
    W_ih = np.asarray(inputs["W_ih"], dtype=np.float32)
    W_hh = np.asarray(inputs["W_hh"], dtype=np.float32)
    b_ih = np.ascontiguousarray(np.asarray(inputs["b_ih"], np.float32))
    b_hh = np.ascontiguousarray(np.asarray(inputs["b_hh"], np.float32))
    fc_W = np.asarray(inputs["fc_W"], dtype=np.float32)
    fc_b = np.asarray(inputs["fc_b"], dtype=np.float32)

    enc_w = wcast(enc)
    attn_wt_w = wcast(attn_W.T)
    in_maps = []
    for c in range(NCORES):
        hs = slice(c * SH, (c + 1) * SH)
        v0 = c * VS
        v1 = min(V, v0 + VS)
        fcw = np.zeros((H, VS), dtype=np.float32)
        fcw[:, :v1 - v0] = fc_W[v0:v1, :].T
        fcb = np.full((VS,), NEG, dtype=np.float32)
        fcb[:v1 - v0] = fc_b[v0:v1]
        in_maps.append({
            "emb": emb_row,
            "h0": h0f,
            "h0s": np.ascontiguousarray(h0f[hs]),
            "attn_wt": attn_wt_w,
            "attn_b": attn_b,
            "enc": enc_w,
            "comb_wt": wcast(comb_W[hs, :].T),
            "comb_b": np.ascontiguousarray(comb_b[hs]),
            "wih_t": wcast(W_ih[:, hs].T),
            "whh_t": wcast(W_hh[:, hs].T),
            "b_ih": b_ih,
            "b_hh": b_hh,
            "fc_wt": wcast(fcw),
            "fc_b": fcb,
        })
    return in_maps


def kernel(**inputs):
    from concourse import bass_utils
    use_bf16 = USE_BF16
    nc = _get_nc(use_bf16)
    in_maps = _prep_in_maps(inputs, use_bf16)
    res = bass_utils.run_bass_kernel_spmd(nc, in_maps, list(range(NCORES)))
    results = res.results
    logits = np.concatenate([results[c]["out_logits"] for c in range(NCORES)])
    out = logits[:V].reshape(1, V).astype(np.float32)
    h1 = results[0]["out_h1"].reshape(1, 1, H).astype(np.float32)
    aw = results[0]["out_aw"].reshape(1, L).astype(np.float32)
    return out, h1, aw


# revision 33
# speedup vs baseline: 1.1713x; 1.1713x over previous
"""Distributed Trainium2 Bass kernel for the single-step attention decoder.

Strategy (8-way tensor parallel, memory-bound matvec workload):
  - emb_table[x] gathered on host (only 1 row of 411MB is needed).
  - attn_W and encoder_outputs replicated (small); comb_W row-sharded (256
    of 2048 output dims); W_ih/W_hh contraction-sharded (aligned with comb's
    row shard); fc_W vocab-sharded (6400 padded rows/core).
  - 2 AllGathers: partial GRU gate pre-activations, and per-core
    (max, sumexp) stats for the distributed log-softmax.
  - Matvecs run on the TensorEngine with the vector chunk stationary and the
    weight tile moving, except fc, where the weight tile is stationary so
    the logits land partition-major for a cheap parallel softmax.
  - Weight shards are pre-transposed on the host so every device DMA is a
    large contiguous burst with the contraction dim on SBUF partitions.
  - Partition-axis relayouts (vector [N] <-> SBUF [128, N/128]) go through
    PE transposes; partition-scatter DMA patterns run at ~3GB/s and are
    avoided everywhere.
  - DMA queue discipline: each engine's DMA stream is FIFO and blocks on
    pool-slot waits, so the fc weight stream gets the sync queue to itself
    (prefetches from t=0); all chain weights go in chain order on the
    scalar queue.
"""
import os
import sys
import numpy as np

for _p in ("/opt/trn_rl_repo",):
    if _p not in sys.path:
        sys.path.append(_p)

V, H, L = 50257, 2048, 512
NCORES = 8
VS = 6400            # per-core padded vocab rows = 128 * 50
VT = VS // 128       # 50 vocab tiles per core
HC = H // 128        # 16 hidden chunks
SH = H // NCORES     # 256: comb row shard / gru contraction shard
G3 = 3 * H           # 6144
GT = G3 // 128       # 48 gate tiles
NEG = -1.0e30

USE_BF16 = os.environ.get("ADK_BF16", "1") == "1"

_CACHE = {}


def _build(use_bf16):
    from concourse import bacc, mybir, tile

    dt = mybir.dt
    f32 = dt.float32
    wdt = dt.bfloat16 if use_bf16 else f32
    A = mybir.AluOpType
    AF = mybir.ActivationFunctionType
    X = mybir.AxisListType.X

    nc = bacc.Bacc("TRN2", target_bir_lowering=False, debug=False,
                   enable_asserts=True, num_devices=NCORES)

    def din(name, shape, d=f32):
        return nc.dram_tensor(name, list(shape), d, kind="ExternalInput")

    emb = din("emb", [H])
    h0 = din("h0", [H])
    h0s = din("h0s", [SH])
    attn_wt = din("attn_wt", [2 * H, L], wdt)   # replicated, [j, L]
    attn_b = din("attn_b", [L])
    enc = din("enc", [L, H], wdt)               # replicated
    comb_wt = din("comb_wt", [2 * H, SH], wdt)  # [j, h-shard]
    comb_b = din("comb_b", [SH])
    wih_t = din("wih_t", [SH, G3], wdt)
    whh_t = din("whh_t", [SH, G3], wdt)
    b_ih = din("b_ih", [G3])
    b_hh = din("b_hh", [G3])
    fc_wt = din("fc_wt", [H, VS], wdt)          # [h, v-shard]
    fc_b = din("fc_b", [VS])

    out_logits = nc.dram_tensor("out_logits", [VS], f32, kind="ExternalOutput")
    out_h1 = nc.dram_tensor("out_h1", [H], f32, kind="ExternalOutput")
    out_aw = nc.dram_tensor("out_aw", [L], f32, kind="ExternalOutput")

    ident_np = np.eye(128, dtype=np.float32)
    rg = [list(range(NCORES))]

    with tile.TileContext(nc) as tc:
        with (
            tc.tile_pool(name="small", bufs=1) as sp,
            tc.tile_pool(name="wpool", bufs=1) as wp,
            tc.tile_pool(name="ps", bufs=3, space="PSUM") as ps,
            tc.tile_pool(name="psc", bufs=3, space="PSUM") as psc,
            tc.tile_pool(name="psl", bufs=1, space="PSUM") as psl,
            tc.tile_pool(name="pss", bufs=1, space="PSUM") as pss,
            tc.tile_pool(name="dram", bufs=1, space="DRAM") as dp,
        ):
            # ---------- constants ----------
            id128 = sp.tile([128, 128], f32, tag="id128")
            id_dram = nc.inline_tensor(ident_np, name="id_dram")
            nc.scalar.dma_start(id128[:], id_dram[:])
            ones128 = sp.tile([128, 1], f32, tag="ones128")
            nc.vector.memset(ones128[:], 1.0)
            onesrow = sp.tile([1, 128], f32, tag="onesrow")
            nc.vector.memset(onesrow[:], 1.0)

            def load_T(src_ap, cols, tag, eng=None):
                """DRAM vector [cols*128] -> SBUF [128, cols] via natural
                load + PE transpose (no partition-scatter DMA)."""
                eng = eng or nc.scalar
                nat = wp.tile([cols, 128], f32, tag="ldT_nat", bufs=4,
                              name="ldT_" + tag)
                eng.dma_start(nat[:], src_ap.rearrange("(c p) -> c p", p=128))
                tp = psc.tile([128, 128], f32, tag="col", name="tp_" + tag)
                nc.tensor.transpose(tp[:, 0:cols], nat[:], id128[:cols, :cols])
                t = sp.tile([128, cols], f32, tag=tag, name=tag)
                nc.vector.tensor_copy(t[:], tp[:, 0:cols])
                return t

            def store_T(dst_ap, src_tile, cols, eng=None):
                """SBUF [128, cols] -> DRAM vector [cols*128] via PE
                transpose + natural store."""
                eng = eng or nc.scalar
                tp = psc.tile([128, 128], f32, tag="col", name="tp_st")
                nc.tensor.transpose(tp[:cols, :], src_tile[:], id128[:])
                nat = wp.tile([cols, 128], f32, tag="stT_nat", bufs=2,
                              name="stT_nat")
                nc.vector.tensor_copy(nat[:], tp[:cols, :])
                eng.dma_start(dst_ap.rearrange("(c p) -> c p", p=128), nat[:])

            attnb_sb = sp.tile([1, L], f32, tag="attnb_sb")
            nc.scalar.dma_start(attnb_sb[:], attn_b[None, :])
            emb_sb = load_T(emb[:], HC, "emb_sb")
            h0_sb = load_T(h0[:], HC, "h0_sb")
            h0s_sb = load_T(h0s[:], 2, "h0s_sb")

            if use_bf16:
                emb_w = sp.tile([128, HC], wdt, tag="emb_w")
                nc.vector.tensor_copy(emb_w[:], emb_sb[:])
                h0_w = sp.tile([128, HC], wdt, tag="h0_w")
                nc.vector.tensor_copy(h0_w[:], h0_sb[:])
                h0s_w = sp.tile([128, 2], wdt, tag="h0s_w")
                nc.vector.tensor_copy(h0s_w[:], h0s_sb[:])
            else:
                emb_w, h0_w, h0s_w = emb_sb, h0_sb, h0s_sb

            # softmax helper used by the final distributed log-softmax
            def pmax_bcast_neg(val_pc, tag):
                """[128,1] per-partition vals -> (max, [128,1] bcast of -max)."""
                tp = pss.tile([1, 128], f32, tag="tp", name="tp_" + tag)
                nc.tensor.transpose(tp[:], val_pc[:], id128[:])
                m = sp.tile([1, 1], f32, tag=tag + "_m")
                nc.vector.tensor_reduce(m[:], tp[:], axis=X, op=A.max)
                negm = sp.tile([1, 1], f32, tag=tag + "_negm")
                nc.scalar.mul(negm[:], m[:], -1.0)
                bc_ps = psc.tile([128, 128], f32, tag="col", name="bc_" + tag)
                nc.tensor.matmul(bc_ps[:, 0:1], onesrow[:], negm[:])
                bc = sp.tile([128, 1], f32, tag=tag + "_bc")
                nc.vector.tensor_copy(bc[:], bc_ps[:, 0:1])
                return m, bc

            # ---------- attention scores, fully replicated (no collective)
            att_ps = ps.tile([1, L], f32, tag="v512", name="att_ps")
            for j in range(32):
                wt = wp.tile([128, L], wdt, tag="attn_wt", bufs=8,
                             name=f"attw{j}")
                nc.scalar.dma_start(wt[:], attn_wt[j * 128:(j + 1) * 128, :])
                lhs = emb_w[:, j:j + 1] if j < HC else h0_w[:, j - HC:j - HC + 1]
                nc.tensor.matmul(att_ps[:], lhs, wt[:],
                                 start=(j == 0), stop=(j == 31))

            # enc + gru weight loads issued now on the chain queue (they
            # depend on nothing, and nothing before them can block)
            enc_tiles = []
            for k in range(4):
                et = wp.tile([128, H], wdt, tag="enc", bufs=4, name=f"enc{k}")
                nc.scalar.dma_start(et[:], enc[k * 128:(k + 1) * 128, :])
                enc_tiles.append(et)
            gru_tiles = {}
            for nm, wten in (("hh", whh_t), ("ih", wih_t)):
                for ck in range(2):
                    gwt = wp.tile([128, G3], wdt, tag="gru", bufs=4,
                                  name=f"gru_{nm}{ck}")
                    nc.scalar.dma_start(gwt[:], wten[ck * 128:(ck + 1) * 128, :])
                    gru_tiles[(nm, ck)] = gwt

            # fc weight stream on its own sync queue, issued after the chain
            # weights so the chain gets the bandwidth head start
            fc_bufs = 7 if use_bf16 else 2
            fc_tiles = []
            for c in range(HC):
                wc = wp.tile([128, VS], wdt, tag="fc", bufs=fc_bufs,
                             name=f"fcw{c}")
                for dd in range(2):
                    nc.sync.dma_start(wc[:, dd * 3200:(dd + 1) * 3200],
                                      fc_wt[c * 128:(c + 1) * 128,
                                            dd * 3200:(dd + 1) * 3200])
                fc_tiles.append(wc)

            # single-lane softmax over [1, 512]
            sc_row = sp.tile([1, L], f32, tag="sc_row")
            nc.vector.tensor_add(sc_row[:], att_ps[:], attnb_sb[:])
            smax = sp.tile([1, 1], f32, tag="smax")
            nc.vector.tensor_reduce(smax[:], sc_row[:], axis=X, op=A.max)
            nsmax = sp.tile([1, 1], f32, tag="nsmax")
            nc.scalar.mul(nsmax[:], smax[:], -1.0)
            aw_row = sp.tile([1, L], f32, tag="aw_row")
            ssum = sp.tile([1, 1], f32, tag="ssum")
            nc.scalar.activation(aw_row[:], sc_row[:], AF.Exp, bias=nsmax[:],
                                 accum_out=ssum[:])
            rsum = sp.tile([1, 1], f32, tag="rsum")
            nc.vector.reciprocal(rsum[:], ssum[:])
            nc.vector.tensor_scalar(aw_row[:], aw_row[:], rsum[:], None,
                                    op0=A.mult)
            nc.scalar.dma_start(out_aw[None, :], aw_row[:])

            # attn weights -> partition-major [128, 4] for the applied matvec
            awT_ps = psc.tile([128, 128], f32, tag="col", name="awT")
            for t in range(4):
                nc.tensor.transpose(awT_ps[:, t:t + 1],
                                    aw_row[:, t * 128:(t + 1) * 128],
                                    id128[:1, :1])
            aw_sb = sp.tile([128, 4], f32, tag="aw_sb")
            nc.vector.tensor_copy(aw_sb[:], awT_ps[:, 0:4])
            if use_bf16:
                aw_w = sp.tile([128, 4], wdt, tag="aw_w")
                nc.vector.tensor_copy(aw_w[:], aw_sb[:])
            else:
                aw_w = aw_sb

            # ---------- GRU partials: the hh half depends only on h0s, so
            # it runs while the attention chain is still in flight
            gp_in = dp.tile([1, 2 * G3], f32, tag="gp_in")
            gp_out = dp.tile([NCORES, 2 * G3], f32, tag="gp_out")

            def gru_half(nm, xw, base):
                for s in range(12):
                    gps = ps.tile([1, 512], f32, tag="v512", name=f"g{nm}{s}")
                    nc.tensor.matmul(gps[:], xw[:, 0:1],
                                     gru_tiles[(nm, 0)][:, s * 512:(s + 1) * 512],
                                     start=True, stop=False)
                    nc.tensor.matmul(gps[:], xw[:, 1:2],
                                     gru_tiles[(nm, 1)][:, s * 512:(s + 1) * 512],
                                     start=False, stop=True)
                    off = base + s * 512
                    stg = wp.tile([1, 512], f32, tag="gpstage", bufs=4,
                                  name=f"stg{nm}{s}")
                    nc.vector.tensor_copy(stg[:], gps[:])
                    nc.gpsimd.dma_start(gp_in[:, off:off + 512], stg[:])

            gru_half("hh", h0s_w, G3)

            # ---------- attn_applied = attn_weight @ enc  (full, replicated)
            app_row = sp.tile([1, H], f32, tag="app_row")
            for s in range(4):
                app_ps = ps.tile([1, 512], f32, tag="v512", name=f"app{s}")
                for k in range(4):
                    nc.tensor.matmul(app_ps[:], aw_w[:, k:k + 1],
                                     enc_tiles[k][:, s * 512:(s + 1) * 512],
                                     start=(k == 0), stop=(k == 3))
                nc.vector.tensor_copy(app_row[:, s * 512:(s + 1) * 512],
                                      app_ps[:])

            # transpose [1, 2048] -> [128, 16]
            appT_ps = psc.tile([128, 128], f32, tag="col", name="appT")
            for t in range(HC):
                nc.tensor.transpose(appT_ps[:, t:t + 1],
                                    app_row[:, t * 128:(t + 1) * 128],
                                    id128[:1, :1])
            app_sb = sp.tile([128, HC], f32, tag="app_sb")
            nc.vector.tensor_copy(app_sb[:], appT_ps[:, 0:HC])
            if use_bf16:
                app_w = sp.tile([128, HC], wdt, tag="app_w")
                nc.vector.tensor_copy(app_w[:], app_sb[:])
            else:
                app_w = app_sb

            # ---------- comb: g_in shard [1, 256] = relu(comb_in @ W.T + b)
            combb_sb = sp.tile([1, SH], f32, tag="combb_sb")
            nc.scalar.dma_start(combb_sb[:], comb_b[None, :])
            comb_ps = ps.tile([1, SH], f32, tag="v512", name="comb_ps")
            for j in range(32):
                ct = wp.tile([128, SH], wdt, tag="comb", bufs=6,
                             name=f"combw{j}")
                nc.scalar.dma_start(ct[:], comb_wt[j * 128:(j + 1) * 128, :])
                lhs = emb_w[:, j:j + 1] if j < HC else app_w[:, j - HC:j - HC + 1]
                nc.tensor.matmul(comb_ps[:], lhs, ct[:],
                                 start=(j == 0), stop=(j == 31))
            gin_row = sp.tile([1, SH], f32, tag="gin_row")
            nc.vector.scalar_tensor_tensor(gin_row[:], comb_ps[:], 1.0,
                                           combb_sb[:], op0=A.mult, op1=A.add)
            nc.scalar.activation(gin_row[:], gin_row[:], AF.Relu)
            gT_ps = psc.tile([128, 128], f32, tag="col", name="gT")
            for t in range(2):
                nc.tensor.transpose(gT_ps[:, t:t + 1],
                                    gin_row[:, t * 128:(t + 1) * 128],
                                    id128[:1, :1])
            gin_sb = sp.tile([128, 2], f32, tag="gin_sb")
            nc.vector.tensor_copy(gin_sb[:], gT_ps[:, 0:2])
            if use_bf16:
                gin_w = sp.tile([128, 2], wdt, tag="gin_w")
                nc.vector.tensor_copy(gin_w[:], gin_sb[:])
            else:
                gin_w = gin_sb

            # ---------- GRU ih partials, then gather
            gru_half("ih", gin_w, 0)

            nc.gpsimd.collective_compute(
                "AllGather", mybir.AluOpType.bypass, replica_groups=rg,
                ins=[gp_in.opt()], outs=[gp_out.opt()])

            # bias relayout (off critical path)
            bias_sb = sp.tile([128, 2 * GT], f32, tag="bias_sb")
            bi = load_T(b_ih[:], GT, "bi_tmp")
            bh = load_T(b_hh[:], GT, "bh_tmp")
            nc.vector.tensor_copy(bias_sb[:, 0:GT], bi[:])
            nc.vector.tensor_copy(bias_sb[:, GT:2 * GT], bh[:])

            # relayout gathered partials: flat [8*12288] = [768, 128]
            # -> 6 natural [128,128] tiles -> PE transpose -> col = r*96 + j
            W2 = 2 * GT
            gg2 = sp.tile([128, NCORES * W2], f32, tag="gg2")
            gpf = gp_out[:].rearrange("r (k p) -> (r k) p", p=128)
            for b in range(6):
                gnat = wp.tile([128, 128], f32, tag="gg_nat", bufs=3,
                               name=f"gg_nat{b}")
                nc.scalar.dma_start(gnat[:], gpf[b * 128:(b + 1) * 128, :])
                ggT = psc.tile([128, 128], f32, tag="col", name=f"ggT{b}")
                nc.tensor.transpose(ggT[:], gnat[:], id128[:])
                nc.vector.tensor_copy(gg2[:, b * 128:(b + 1) * 128], ggT[:])
            gsum = sp.tile([128, W2], f32, tag="gsum")
            nc.vector.tensor_add(gsum[:], gg2[:, 0:W2], gg2[:, W2:2 * W2])
            for r in range(2, NCORES):
                nc.vector.tensor_add(gsum[:], gsum[:],
                                     gg2[:, r * W2:(r + 1) * W2])
            nc.vector.tensor_add(gsum[:], gsum[:], bias_sb[:])

            # gates: [ir iz in | hr hz hn] at 16-col blocks
            ir, iz, inn = gsum[:, 0:16], gsum[:, 16:32], gsum[:, 32:48]
            hr, hz, hn = gsum[:, 48:64], gsum[:, 64:80], gsum[:, 80:96]
            r_sb = sp.tile([128, HC], f32, tag="r_sb")
            nc.vector.tensor_add(r_sb[:], ir, hr)
            nc.scalar.activation(r_sb[:], r_sb[:], AF.Sigmoid)
            z_sb = sp.tile([128, HC], f32, tag="z_sb")
            nc.vector.tensor_add(z_sb[:], iz, hz)
            nc.scalar.activation(z_sb[:], z_sb[:], AF.Sigmoid)
            n_sb = sp.tile([128, HC], f32, tag="n_sb")
            nc.vector.tensor_mul(n_sb[:], r_sb[:], hn)
            nc.vector.tensor_add(n_sb[:], n_sb[:], inn)
            nc.scalar.activation(n_sb[:], n_sb[:], AF.Tanh)
            h1_sb = sp.tile([128, HC], f32, tag="h1_sb")
            nc.vector.tensor_sub(h1_sb[:], h0_sb[:], n_sb[:])
            nc.vector.tensor_mul(h1_sb[:], z_sb[:], h1_sb[:])
            nc.vector.tensor_add(h1_sb[:], n_sb[:], h1_sb[:])
            store_T(out_h1[:], h1_sb, HC)
            if use_bf16:
                h1_w = sp.tile([128, HC], wdt, tag="h1_w")
                nc.vector.tensor_copy(h1_w[:], h1_sb[:])
            else:
                h1_w = h1_sb

            # ---------- fc: logits [128, 50] vocab shard
            fcb_sb = load_T(fc_b[:], VT, "fcb_sb")
            lg_ps = psl.tile([128, VT], f32, tag="lg")
            for c in range(HC):
                wc = fc_tiles[c]
                for t in range(VT):
                    # NOTE: start=True clears the accumulate-state of the
                    # whole PSUM bank, so only the very first matmul of the
                    # fc phase may set it; each column's first start=False
                    # write then overwrites (bits clear) and later ones add.
                    nc.tensor.matmul(lg_ps[:, t:t + 1],
                                     wc[:, t * 128:(t + 1) * 128],
                                     h1_w[:, c:c + 1],
                                     start=(c == 0 and t == 0),
                                     stop=(c == HC - 1 and t == VT - 1),
                                     skip_group_check=True)

            lg_sb = sp.tile([128, VT], f32, tag="lg_sb")
            nc.vector.tensor_add(lg_sb[:], lg_ps[:], fcb_sb[:])

            # local softmax stats
            rmax2 = sp.tile([128, 1], f32, tag="rmax2")
            nc.vector.tensor_reduce(rmax2[:], lg_sb[:], axis=X, op=A.max)
            m_sb, nm2_bc = pmax_bcast_neg(rmax2, "fc")
            ex2 = sp.tile([128, VT], f32, tag="ex2")
            se2 = sp.tile([128, 1], f32, tag="se2")
            nc.scalar.activation(ex2[:], lg_sb[:], AF.Exp, bias=nm2_bc[:],
                                 accum_out=se2[:])
            s_ps = pss.tile([1, 128], f32, tag="tp", name="s_ps")
            nc.tensor.matmul(s_ps[:1, 0:1], se2[:], ones128[:])
            stats_sb = sp.tile([1, 2], f32, tag="stats_sb")
            nc.scalar.copy(stats_sb[:, 0:1], m_sb[:])
            nc.scalar.copy(stats_sb[:, 1:2], s_ps[:1, 0:1])

            st_in = dp.tile([1, 2], f32, tag="st_in")
            st_out = dp.tile([NCORES, 2], f32, tag="st_out")
            nc.scalar.dma_start(st_in[:], stats_sb[:])
            nc.gpsimd.collective_compute(
                "AllGather", mybir.AluOpType.bypass, replica_groups=rg,
                ins=[st_in.opt()], outs=[st_out.opt()])
            sts = sp.tile([1, NCORES, 2], f32, tag="sts")
            nc.scalar.dma_start(sts[:], st_out[None, :, :])

            # transpose the biased logits while the stats AllGather runs
            lgT_ps = psc.tile([128, 128], f32, tag="col", name="lgT_ps")
            nc.tensor.transpose(lgT_ps[:VT, :], lg_sb[:], id128[:])
            lgT = sp.tile([VT, 128], f32, tag="lgT")
            nc.vector.tensor_copy(lgT[:], lgT_ps[:VT, :])

            gm = sp.tile([1, 1], f32, tag="gm")
            nc.vector.tensor_reduce(gm[:], sts[:, :, 0], axis=X, op=A.max)
            ngm = sp.tile([1, 1], f32, tag="ngm")
            nc.scalar.mul(ngm[:], gm[:], -1.0)
            ee = sp.tile([1, NCORES], f32, tag="ee")
            nc.scalar.activation(ee[:], sts[:, :, 0], AF.Exp, bias=ngm[:])
            nc.vector.tensor_mul(ee[:], ee[:], sts[:, :, 1])
            gs = sp.tile([1, 1], f32, tag="gs")
            nc.vector.tensor_reduce(gs[:], ee[:], axis=X, op=A.add)
            lgs = sp.tile([1, 1], f32, tag="lgs")
            nc.scalar.activation(lgs[:], gs[:], AF.Ln)
            cv = sp.tile([1, 1], f32, tag="cv")
            nc.vector.tensor_add(cv[:], gm[:], lgs[:])
            ncv = sp.tile([1, 1], f32, tag="ncv")
            nc.scalar.mul(ncv[:], cv[:], -1.0)
            ncv_ps = psc.tile([128, 128], f32, tag="col", name="ncv_ps")
            nc.tensor.matmul(ncv_ps[:VT, 0:1], onesrow[:1, 0:VT], ncv[:])
            ncv_bc = sp.tile([VT, 1], f32, tag="ncv_bc")
            nc.vector.tensor_copy(ncv_bc[:], ncv_ps[:VT, 0:1])
            fin = sp.tile([VT, 128], f32, tag="fin")
            nc.vector.tensor_scalar(fin[:], lgT[:], ncv_bc[:], None,
                                    op0=A.add)
            nc.sync.dma_start(out_logits[:].rearrange("(c p) -> c p", p=128),
                              fin[:])

    nc.compile()
    return nc


def _get_nc(use_bf16):
    if use_bf16 not in _CACHE:
        _CACHE[use_bf16] = _build(use_bf16)
    return _CACHE[use_bf16]


def _prep_in_maps(inputs, use_bf16):
    import ml_dtypes
    wnp = ml_dtypes.bfloat16 if use_bf16 else np.float32

    def wcast(a):
        return np.ascontiguousarray(a, dtype=np.float32).astype(wnp) \
            if use_bf16 else np.ascontiguousarray(a, dtype=np.float32)

    x = np.asarray(inputs["x"]).reshape(-1)
    idx = int(x[0])
    emb_tab = np.asarray(inputs["emb_table"], dtype=np.float32)
    emb_row = np.ascontiguousarray(emb_tab[idx])
    h0f = np.asarray(inputs["hidden"], dtype=np.float32).reshape(H)
    attn_W = np.asarray(inputs["attn_W"], dtype=np.float32)
    attn_b = np.ascontiguousarray(np.asarray(inputs["attn_b"], np.float32))
    enc = np.asarray(inputs["encoder_outputs"], dtype=np.float32)
    comb_W = np.asarray(inputs["comb_W"], dtype=np.float32)
    comb_b = np.asarray(inputs["comb_b"], dtype=np.float32)
    W_ih = np.asarray(inputs["W_ih"], dtype=np.float32)
    W_hh = np.asarray(inputs["W_hh"], dtype=np.float32)
    b_ih = np.ascontiguousarray(np.asarray(inputs["b_ih"], np.float32))
    b_hh = np.ascontiguousarray(np.asarray(inputs["b_hh"], np.float32))
    fc_W = np.asarray(inputs["fc_W"], dtype=np.float32)
    fc_b = np.asarray(inputs["fc_b"], dtype=np.float32)

    enc_w = wcast(enc)
    attn_wt_w = wcast(attn_W.T)
    in_maps = []
    for c in range(NCORES):
        hs = slice(c * SH, (c + 1) * SH)
        v0 = c * VS
        v1 = min(V, v0 + VS)
        fcw = np.zeros((H, VS), dtype=np.float32)
        fcw[:, :v1 - v0] = fc_W[v0:v1, :].T
        fcb = np.full((VS,), NEG, dtype=np.float32)
        fcb[:v1 - v0] = fc_b[v0:v1]
        in_maps.append({
            "emb": emb_row,
            "h0": h0f,
            "h0s": np.ascontiguousarray(h0f[hs]),
            "attn_wt": attn_wt_w,
            "attn_b": attn_b,
            "enc": enc_w,
            "comb_wt": wcast(comb_W[hs, :].T),
            "comb_b": np.ascontiguousarray(comb_b[hs]),
            "wih_t": wcast(W_ih[:, hs].T),
            "whh_t": wcast(W_hh[:, hs].T),
            "b_ih": b_ih,
            "b_hh": b_hh,
            "fc_wt": wcast(fcw),
            "fc_b": fcb,
        })
    return in_maps


def kernel(**inputs):
    from concourse import bass_utils
    use_bf16 = USE_BF16
    nc = _get_nc(use_bf16)
    in_maps = _prep_in_maps(inputs, use_bf16)
    res = bass_utils.run_bass_kernel_spmd(nc, in_maps, list(range(NCORES)))
    results = res.results
    logits = np.concatenate([results[c]["out_logits"] for c in range(NCORES)])
    out = logits[:V].reshape(1, V).astype(np.float32)
    h1 = results[0]["out_h1"].reshape(1, 1, H).astype(np.float32)
    aw = results[0]["out_aw"].reshape(1, L).astype(np.float32)
    return out, h1, aw
